# revision 48
# baseline (speedup 1.0000x reference)
"""CliffordLinear (Cl(3,0)) Trainium2 kernel.

Math: Cl(3,0) is isomorphic to the 2x2 complex matrices via the Pauli
representation phi.  The reference's per-channel Clifford contraction maps to
one complex matrix product  phi(Out)[:,c] = phi(W) @ phi(X)[:,c]  per output
column c in {0,1}, i.e. for each c the real [b x 512] panels XRe, XIm against
the real 512x512 planes R, I of phi(W):

    Re = XRe@R - XIm@I          Im = XRe@I + XIm@R

Gauss's 3-multiplication trick computes both from three products

    G1 = XRe@R   G2 = XIm@I   G3 = (XRe+XIm)@(R+I)
    Re = G1 - G2                Im = G3 - G1 - G2

which is 12 accumulation k-tiles per (batch-tile, c) instead of 16 -- a 25%
Tensor-engine saving (12.9G real MACs vs 17.2G for the 4-mult scheme, vs
34.4G naive blades).  The XRe+XIm panel and R+I plane are precomputed on the
host and shipped, so the trick costs no device arithmetic on the input side.

All operands move as bf16 (half the HBM traffic of fp32; matmuls run at the
same PE rate as fp32r).  Per (bt, c): PE accumulates G1/G2/G3 into three
one-bank PSUM tiles; ScalarE evicts each to bf16 SBUF; DVE does the Gauss
recombine and the inverse-Pauli blade butterfly entirely in packed bf16
(2x DVE rate), writing a blade-major stage tile that DMAs out as bf16.
Weight columns are r-major (col = r*256 + o) so every butterfly operand is
contiguous; the host unshuffles the blade-major output.

Derived operands never touch HBM: the XSum = XRe+XIm panel (1/3 of x) and
the R+I plane are summed on the DVE.  Loads ride the SP/HWDGE queue, steady
stores ride gpsimd's SWDGE queue (Pool) so a store's sem wait can never
head-of-line block a load.  The last bt phases its eviction Re-then-Im with
G3c1 split into two half-column PSUM groups, so only a ~400-col chain
trails the final matmul.

Sharding: data-parallel over batch (1024 rows/core); weights replicated.
Per-core HBM: 4.2 MB x + 1.05 MB w in, 4.2 MB out (~26 us at 360 B/ns),
under the ~41 us PE floor.  Cost-model time: 51.9 us/core (from 70.4 us).
"""

import sys

sys.path.insert(0, "/opt/trn_rl_repo")

import numpy as np

import concourse.bass as bass  # noqa: F401  (registers lowerings)
import concourse.mybir as mybir
import concourse.tile as tile
from concourse import bacc
from concourse.bass_utils import run_bass_kernel_spmd

N_CORES = 8
B, CIN, COUT, NB = 8192, 256, 256, 8
BS = B // N_CORES          # 1024 batch rows per core
BT = BS // 128             # 8 b-tiles
KP = 2 * CIN               # 512 contraction rows per panel (i,m)
PKT = KP // 128            # 4 k-tiles per panel
XKT = 2 * PKT              # 8 x k-tiles per c (Re | Im; Sum panel is
                           # recomputed on-device, saving 1/3 of x HBM)
OUTW = COUT * NB           # 2048 output cols (blade-major: col = blade*256+o)

_cached = {}

N_WARM = 2         # PE p-state ramp matmuls before real data lands
PS_DB0 = 2         # ring depth for PSUM tag g00 (first group of each bt)
PS_DB1 = 1         # ring depth for PSUM tag g10 (g3b needs the 8th bank)


def _rw_ap(base, off, dims):
    """Clone `base` keeping its leading (partition/row) dim, replacing the
    free dims with `dims` [(step, num), ...] and adding `off` elements."""
    a = base.copy()
    part = a.ap.to_list()[0]
    v = a.ap
    v.clear()
    v.extend([tuple(part)] + [tuple(d) for d in dims])
    a.offset = a.offset + off
    return a


def _build_nc():
    bf = mybir.dt.bfloat16
    f32 = mybir.dt.float32
    nc = bacc.Bacc("TRN2", target_bir_lowering=False, debug=False,
                   num_devices=N_CORES)
    # x panels per phi-column c: [bt, p, kk*128 + b] with contraction row
    # kappa = k*128 + p inside panel kk//4 (0:XRe, 1:XIm, 2:XRe+XIm)
    xt = [nc.dram_tensor(f"xt{c}", [BT, 128, XKT * 128], bf,
                         kind="ExternalInput") for c in range(2)]
    # weight planes [R, I]: rows kappa=(i,m), cols r-major r*256+o.
    # The Gauss R+I plane is summed on-device (saves startup HBM traffic).
    wt = nc.dram_tensor("wt", [2, KP, 512], bf, kind="ExternalInput")
    out = nc.dram_tensor("out", [BS, OUTW], bf, kind="ExternalOutput")

    with tile.TileContext(nc) as tc:
        with tc.tile_pool(name="wpool", bufs=1) as wpool, \
             tc.tile_pool(name="xpool", bufs=3) as xpool, \
             tc.tile_pool(name="epool", bufs=2) as epool, \
             tc.tile_pool(name="pspool", bufs=1, space="PSUM") as pspool:
            # PE warmup on zeros: ramps the PE p-state during the initial DMA
            # wait so real matmuls start at full clock.
            warm_in = wpool.tile([128, 384], bf, tag="warm_in")
            nc.vector.memset(warm_in[:].bitcast(mybir.dt.uint32), 0)
            # warm_ps shares the g00 ring (retired after warmup, so the
            # spare PSUM banks go to double-buffering g00/g10 instead)
            warm_ps = pspool.tile([128, 512], f32, tag="g00", bufs=PS_DB0)
            for _ in range(N_WARM):
                nc.tensor.matmul(warm_ps[:, 0:256], warm_in[:, :128],
                                 warm_in[:, 128:384], start=True, stop=True)

            # Startup: interleave weight k-tiles with bt0's x panels in PE
            # consumption order (bt0 runs G-major: G1 both c, then G2, G3).
            # The R+I plane never moves over HBM -- DVE sums it from the R
            # and I tiles while the PE chews on G1/G2.
            w_t = [[None] * PKT for _ in range(3)]
            x0_p = [[None] * 3 for _ in range(2)]   # [c][panel] tiles for bt0

            def _w_kgrp(p, k0, nk):
                # one DMA covering k-tiles [k0, k0+nk) of plane p into
                # separate 512-col views of one tile: src rows kappa =
                # k*128 + p_low -> dest [p_low, k, col]
                t = wpool.tile([128, nk * 512], bf, tag=f"w{p}g{k0}",
                               name=f"w{p}g{k0}")
                for k in range(nk):
                    w_t[p][k0 + k] = t[:, k * 512:(k + 1) * 512]
                src = wt[p].copy()
                spart = tuple(src.ap.to_list()[0])
                v = src.ap
                v.clear()
                v.extend([(spart[0], 128), (spart[0] * 128, nk), (1, 512)])
                src.offset = src.offset + k0 * 128 * spart[0]
                nc.sync.dma_start(_rw_ap(t[:], 0, [(512, nk), (1, 512)]), src)

            def wrhs(p, k):
                return w_t[p][k]

            def _x0_dma(c, p):
                # bt0's x panels ride the gpsimd SWDGE queue: Pool generates
                # descriptors in parallel with HWDGE, halving the startup
                # per-DMA overhead serialization.
                x0_p[c][p] = xpool.tile([128, 512], bf, tag=f"x0{c}{p}",
                                        bufs=1, name=f"x0{c}{p}")
                nc.gpsimd.dma_start(x0_p[c][p][:],
                                    xt[c][0][:, p * 512:(p + 1) * 512])

            _x0_dma(0, 0)
            _w_kgrp(0, 0, 1)
            _x0_dma(1, 0)
            _w_kgrp(0, 1, 3)
            _x0_dma(0, 1)
            for k in range(PKT):
                _w_kgrp(1, k, 1)
            _x0_dma(1, 1)

            def _xsum(c, src_re, src_im):
                # on-device XSum panel for the G3 product
                xs = xpool.tile([128, 512], bf, tag=f"xs{c}", bufs=2,
                                name=f"xs{c}")
                nc.vector.tensor_add(xs[:], src_re, src_im)
                return xs

            xs0_c0 = _xsum(0, x0_p[0][0][:], x0_p[0][1][:])
            for k in range(PKT):
                w_t[2][k] = wpool.tile([128, 512], bf, tag=f"w2{k}",
                                       name=f"w2{k}")
                nc.vector.tensor_add(w_t[2][k][:], w_t[0][k], w_t[1][k])
            xs0_c1 = _xsum(1, x0_p[1][0][:], x0_p[1][1][:])

            def _mk_xlhs(panels, xs):
                # panels[c][p] for p in {0,1} are 512-col APs; xs[c] the sum
                def xlhs(c, p, k):
                    if p == 2:
                        return xs[c][:, k * 128:(k + 1) * 128]
                    return panels[c][p][:, k * 128:(k + 1) * 128]
                return xlhs

            cur_xlhs = _mk_xlhs(x0_p, [xs0_c0, xs0_c1])

            for bt in range(BT):
                # Prefetch bt+1's x one full window ahead: DMA + the
                # on-device XSum add, so neither is ever on the PE's path.
                if bt + 1 < BT:
                    nxt = []
                    for c in range(2):
                        t = xpool.tile([128, XKT * 128], bf, tag=f"x{c}",
                                       name=f"x{c}")
                        nc.sync.dma_start(t[:], xt[c][bt + 1])
                        nxt.append(t)
                    panels = [[t[:, 0:512], t[:, 512:1024]] for t in nxt]
                    xs = [_xsum(c, panels[c][0], panels[c][1])
                          for c in range(2)]
                    next_xlhs = _mk_xlhs(panels, xs)

                xlhs = cur_xlhs
                if bt + 1 < BT:
                    cur_xlhs = next_xlhs

                last = bt == BT - 1
                Gs = [[None] * 3, [None] * 3]

                def emit_mm(c, p, half=None, Gs=Gs, xlhs=xlhs):
                    bufs = (PS_DB0 if c == 0 else PS_DB1) if p == 0 else 1
                    g = pspool.tile([128, 512], f32, tag=f"g{c}{p}",
                                    name=f"g{c}{p}", bufs=bufs)
                    for k in range(PKT):
                        rhs = wrhs(p, k) if half is None else \
                            wrhs(p, k)[:, 256 * half:256 * (half + 1)]
                        o = g[:] if half is None else g[:, 0:256]
                        nc.tensor.matmul(o, xlhs(c, p, k), rhs,
                                         start=(k == 0), stop=(k == PKT - 1))
                    Gs[c][p] = g

                g3b = None
                if bt == 0 or last:
                    # G-major: bt0 matches the startup DMA arrival order; the
                    # last bt wants G1/G2 stopped early so the Re-blade
                    # eviction and stores run under the G3 matmuls.
                    for p in range(2):
                        for c in range(2):
                            emit_mm(c, p)
                    emit_mm(0, 2)
                    if last:
                        # G3c1 as two half-column PSUM groups: the first
                        # half's Im eviction+store pipelines under the
                        # second half's matmuls.
                        emit_mm(1, 2, half=0)
                        g3b = pspool.tile([128, 512], f32, tag="g3b")
                        for k in range(PKT):
                            nc.tensor.matmul(g3b[:, 0:256], xlhs(1, 2, k),
                                             wrhs(2, k)[:, 256:512],
                                             start=(k == 0), stop=(k == PKT - 1))
                    else:
                        emit_mm(1, 2)
                else:
                    for c in range(2):
                        for p in range(3):
                            emit_mm(c, p)

                # Eviction.  ScalarE copies PSUM -> bf16 SBUF (DVE reads at
                # most one PSUM operand, and all-bf16 doubles the DVE rate);
                # DVE does the Gauss recombine into t = [Re 512 | Im 512]
                # (r-major halves: A/C = r0/r1 of c0, B/D = r0/r1 of c1) and
                # the inverse-Pauli butterfly into the blade-major stage:
                #   x0 = ReA+ReD  x4 = ReA-ReD  x7 = ImA+ImD  x3 = ImA-ImD
                #   x1 = ReC+ReB  x5 = ReC-ReB  x6 = ImC+ImB  x2 = ImC-ImB
                add, sub = nc.vector.tensor_add, nc.vector.tensor_sub
                inner = (1, 256)
                stage = epool.tile([128, OUTW], bf, tag="stage")
                orows = out[bt * 128:(bt + 1) * 128, 0:OUTW]
                # ACT copies in matmul-stop order so no copy head-of-line
                # blocks an already-stopped G behind it on the in-order ACT
                # engine (stops are G-major on bt0/last, c-major otherwise).
                t_c, u_c = [], []
                gs_c = [[None] * 3, [None] * 3]
                np_copy = 2 if last else 3
                order = [(c, p) for p in range(np_copy) for c in range(2)] \
                    if (bt == 0 or last) else \
                    [(c, p) for c in range(2) for p in range(np_copy)]
                for c, p in order:
                    s = epool.tile([128, 512], bf, tag=f"gs{c}{p}",
                                   name=f"gs{c}{p}")
                    nc.scalar.copy(s[:], Gs[c][p][:])
                    gs_c[c][p] = s
                for c in range(2):
                    gs = gs_c[c]
                    t = epool.tile([128, 1024], bf, tag=f"t{c}", name=f"t{c}")
                    u = epool.tile([128, 512], bf, tag=f"u{c}", name=f"u{c}")
                    nc.vector.tensor_sub(t[:, 0:512], gs[0][:], gs[1][:])
                    nc.vector.tensor_add(u[:], gs[0][:], gs[1][:])
                    if not last:
                        nc.vector.tensor_sub(t[:, 512:1024], gs[2][:], u[:])
                    t_c.append(t)
                    u_c.append(u)

                if not last:
                    # Dual-blade butterfly ops; j picks the Re/Im halves.
                    add(_rw_ap(stage[:], 0 * 256, [(1792, 2), inner]),
                        _rw_ap(t_c[0][:], 0, [(512, 2), inner]),
                        _rw_ap(t_c[1][:], 256, [(512, 2), inner]))
                    sub(_rw_ap(stage[:], 4 * 256, [(-256, 2), inner]),
                        _rw_ap(t_c[0][:], 0, [(512, 2), inner]),
                        _rw_ap(t_c[1][:], 256, [(512, 2), inner]))
                    add(_rw_ap(stage[:], 1 * 256, [(1280, 2), inner]),
                        _rw_ap(t_c[0][:], 256, [(512, 2), inner]),
                        _rw_ap(t_c[1][:], 0, [(512, 2), inner]))
                    sub(_rw_ap(stage[:], 5 * 256, [(-768, 2), inner]),
                        _rw_ap(t_c[0][:], 256, [(512, 2), inner]),
                        _rw_ap(t_c[1][:], 0, [(512, 2), inner]))
                    # Steady stores ride gpsimd's SWDGE queue: the sem wait
                    # parks on the otherwise-idle Pool SEQ, so the SP load
                    # queue never stalls behind a store.
                    nc.gpsimd.dma_start(orows, stage[:])
                else:
                    # Re/Im-phased tail: Re blades (j duals (x0,x1), (x4,x5))
                    # need only G1/G2 -- they evict and store while the G3
                    # matmuls still run.  Only the Im blades wait on G3.
                    add(_rw_ap(stage[:], 0, [(256, 2), inner]),
                        _rw_ap(t_c[0][:], 0, [(256, 2), inner]),
                        _rw_ap(t_c[1][:], 256, [(-256, 2), inner]))
                    sub(_rw_ap(stage[:], 1024, [(256, 2), inner]),
                        _rw_ap(t_c[0][:], 0, [(256, 2), inner]),
                        _rw_ap(t_c[1][:], 256, [(-256, 2), inner]))
                    nc.gpsimd.dma_start(
                        _rw_ap(orows, 0, [(1024, 2), (1, 512)]),
                        _rw_ap(stage[:], 0, [(1024, 2), (1, 512)]))
                    # Im phase.  c0 full-width (runs under G3c1's matmuls);
                    # c1 in pipelined halves a (cols 0:256 = ImB) and b
                    # (256:512 = ImD, the g3b bank).  Blade singles:
                    #   x2 = ImC-ImB   x6 = ImC+ImB   (a half)
                    #   x3 = ImA-ImD   x7 = ImA+ImD   (b half)
                    s0 = epool.tile([128, 512], bf, tag="gs02", name="gs02")
                    nc.scalar.copy(s0[:], Gs[0][2][:])
                    sa = epool.tile([128, 512], bf, tag="gs12", name="gs12")
                    nc.scalar.copy(sa[:, 0:256], Gs[1][2][:, 0:256])
                    nc.scalar.copy(sa[:, 256:512], g3b[:, 0:256])
                    nc.vector.tensor_sub(t_c[0][:, 512:1024], s0[:], u_c[0][:])
                    nc.vector.tensor_sub(t_c[1][:, 512:768], sa[:, 0:256],
                                         u_c[1][:, 0:256])
                    sub(stage[:, 512:768], t_c[0][:, 768:1024],
                        t_c[1][:, 512:768])
                    add(stage[:, 1536:1792], t_c[0][:, 768:1024],
                        t_c[1][:, 512:768])
                    nc.scalar.dma_start(
                        _rw_ap(orows, 512, [(1024, 2), inner]),
                        _rw_ap(stage[:], 512, [(1024, 2), inner]))
                    nc.vector.tensor_sub(t_c[1][:, 768:1024], sa[:, 256:512],
                                         u_c[1][:, 256:512])
                    sub(stage[:, 768:1024], t_c[0][:, 512:768],
                        t_c[1][:, 768:1024])
                    add(stage[:, 1792:2048], t_c[0][:, 512:768],
                        t_c[1][:, 768:1024])
                    nc.sync.dma_start(
                        _rw_ap(orows, 768, [(1024, 2), inner]),
                        _rw_ap(stage[:], 768, [(1024, 2), inner]))
    nc.finalize()
    return nc


def _pauli_parts(v):
    """v[..., 8] -> c0, c1 of shape [..., 2(m/r), 2(reim)]: the c-th column
    (Re, Im) of phi(v).  phi entries: A=P00=(v0+v4)+i(v3+v7),
    B=P01=(v1-v5)+i(v6-v2), C=P10=(v1+v5)+i(v6+v2), D=P11=(v0-v4)+i(v7-v3)."""
    c0 = np.empty(v.shape[:-1] + (2, 2), dtype=v.dtype)
    c1 = np.empty_like(c0)
    v0, v1, v2, v3, v4, v5, v6, v7 = (v[..., a] for a in range(8))
    c0[..., 0, 0] = v0 + v4   # Re A
    c0[..., 0, 1] = v3 + v7   # Im A
    c0[..., 1, 0] = v1 + v5   # Re C
    c0[..., 1, 1] = v6 + v2   # Im C
    c1[..., 0, 0] = v1 - v5   # Re B
    c1[..., 0, 1] = v6 - v2   # Im B
    c1[..., 1, 0] = v0 - v4   # Re D
    c1[..., 1, 1] = v7 - v3   # Im D
    return c0, c1


def _np_bf16():
    return mybir.dt.np(mybir.dt.bfloat16)


def _prep_w(weight):
    """weight [COUT, CIN, 8] -> [2, 512, 512] planes [R, I] of phi(W)[r,m],
    rows (i,m), cols r-major (col = r*256 + o), 0.5 folded.  The Gauss R+I
    plane is summed on-device."""
    w = weight.astype(np.float32)
    cw0, cw1 = _pauli_parts(w)    # cw_m[o, i, r, (re,im)] = phi(W[o,i])[r,m]
    R = np.empty((CIN, 2, 2, COUT), np.float32)   # [(i,m),(r,o)]
    I = np.empty_like(R)
    for m, cm in ((0, cw0), (1, cw1)):
        for r in range(2):
            R[:, m, r, :] = 0.5 * cm[:, :, r, 0].T
            I[:, m, r, :] = 0.5 * cm[:, :, r, 1].T
    Rm = R.reshape(KP, 512)
    Im_ = I.reshape(KP, 512)
    return np.ascontiguousarray(
        np.stack([Rm, Im_], axis=0)).astype(_np_bf16())


def _prep_x(x):
    """x [B, CIN, 8] -> per-core arrays [N_CORES][BT, 128, XKT*128] bf16 for
    c in {0,1}: panels [XRe | XIm | XRe+XIm], device layout [bt, p, kk, b]
    with kappa = k*128 + p, col = kk*128 + b."""
    xf = x.astype(np.float32)
    c0, c1 = _pauli_parts(xf)          # [B, CIN, m, reim]
    outs = []
    for arr in (c0, c1):
        re = arr[..., 0].reshape(B, KP)          # kappa = i*2+m
        im = arr[..., 1].reshape(B, KP)
        panels = np.concatenate([re, im], axis=1)            # col = kk*128+p
        a = panels.reshape(N_CORES, BT, 128, XKT, 128)  # [core, bt, b, kk, p]
        a = a.transpose(0, 1, 4, 3, 2)                  # [core, bt, p, kk, b]
        outs.append(np.ascontiguousarray(
            a.reshape(N_CORES, BT, 128, XKT * 128)).astype(_np_bf16()))
    return outs


def kernel(x, weight, bias, cayley):
    assert x.shape == (B, CIN, NB) and weight.shape == (COUT, CIN, NB)
    if "nc" not in _cached:
        _cached["nc"] = _build_nc()
    nc = _cached["nc"]

    xp = _prep_x(np.asarray(x))
    wp = _prep_w(np.asarray(weight))
    in_maps = [{"xt0": xp[0][c], "xt1": xp[1][c], "wt": wp}
               for c in range(N_CORES)]
    res = run_bass_kernel_spmd(nc, in_maps, core_ids=list(range(N_CORES)))
    out = np.concatenate(
        [np.asarray(res.results[c]["out"]).astype(np.float32)
         for c in range(N_CORES)], axis=0)
    # cols are blade-major (blade*256 + o) -> [B, COUT, NB]
    out = out.reshape(B, NB, COUT).transpose(0, 2, 1)
    out = out + np.asarray(bias, np.float32)[None]
    return np.ascontiguousarray(out.astype(np.float32))


# revision 54
# speedup vs baseline: 1.0066x; 1.0066x over previous
"""CliffordLinear (Cl(3,0)) Trainium2 kernel.

Math: Cl(3,0) is isomorphic to the 2x2 complex matrices via the Pauli
representation phi.  The reference's per-channel Clifford contraction maps to
one complex matrix product  phi(Out)[:,c] = phi(W) @ phi(X)[:,c]  per output
column c in {0,1}, i.e. for each c the real [b x 512] panels XRe, XIm against
the real 512x512 planes R, I of phi(W):

    Re = XRe@R - XIm@I          Im = XRe@I + XIm@R

Gauss's 3-multiplication trick computes both from three products

    G1 = XRe@R   G2 = XIm@I   G3 = (XRe+XIm)@(R+I)
    Re = G1 - G2                Im = G3 - G1 - G2

which is 12 accumulation k-tiles per (batch-tile, c) instead of 16 -- a 25%
Tensor-engine saving (12.9G real MACs vs 17.2G for the 4-mult scheme, vs
34.4G naive blades).  The XRe+XIm panel and R+I plane are precomputed on the
host and shipped, so the trick costs no device arithmetic on the input side.

All operands move as bf16 (half the HBM traffic of fp32; matmuls run at the
same PE rate as fp32r).  Per (bt, c): PE accumulates G1/G2/G3 into three
one-bank PSUM tiles; ScalarE evicts each to bf16 SBUF; DVE does the Gauss
recombine and the inverse-Pauli blade butterfly entirely in packed bf16
(2x DVE rate), writing a blade-major stage tile that DMAs out as bf16.
Weight columns are r-major (col = r*256 + o) so every butterfly operand is
contiguous; the host unshuffles the blade-major output.

Derived operands never touch HBM: the XSum = XRe+XIm panel (1/3 of x) and
the R+I plane are summed on the DVE.  Loads ride the SP/HWDGE queue, steady
stores ride gpsimd's SWDGE queue (Pool) so a store's sem wait can never
head-of-line block a load.  The last bt phases its eviction Re-then-Im with
G3c1 split into two half-column PSUM groups, so only a ~400-col chain
trails the final matmul.

Sharding: data-parallel over batch (1024 rows/core); weights replicated.
Per-core HBM: 4.2 MB x + 1.05 MB w in, 4.2 MB out (~26 us at 360 B/ns),
under the ~41 us PE floor.  Cost-model time: 51.9 us/core (from 70.4 us).
"""

import sys

sys.path.insert(0, "/opt/trn_rl_repo")

import numpy as np

import concourse.bass as bass  # noqa: F401  (registers lowerings)
import concourse.mybir as mybir
import concourse.tile as tile
from concourse import bacc
from concourse.bass_utils import run_bass_kernel_spmd

N_CORES = 8
B, CIN, COUT, NB = 8192, 256, 256, 8
BS = B // N_CORES          # 1024 batch rows per core
BT = BS // 128             # 8 b-tiles
KP = 2 * CIN               # 512 contraction rows per panel (i,m)
PKT = KP // 128            # 4 k-tiles per panel
XKT = 2 * PKT              # 8 x k-tiles per c (Re | Im; Sum panel is
                           # recomputed on-device, saving 1/3 of x HBM)
OUTW = COUT * NB           # 2048 output cols (blade-major: col = blade*256+o)

_cached = {}

N_WARM = 2         # PE p-state ramp matmuls before real data lands
PS_DB0 = 2         # ring depth for PSUM tag g00 (first group of each bt)
PS_DB1 = 1         # ring depth for PSUM tag g10 (g3b needs the 8th bank)


def _rw_ap(base, off, dims):
    """Clone `base` keeping its leading (partition/row) dim, replacing the
    free dims with `dims` [(step, num), ...] and adding `off` elements."""
    a = base.copy()
    part = a.ap.to_list()[0]
    v = a.ap
    v.clear()
    v.extend([tuple(part)] + [tuple(d) for d in dims])
    a.offset = a.offset + off
    return a


def _build_nc():
    bf = mybir.dt.bfloat16
    f32 = mybir.dt.float32
    nc = bacc.Bacc("TRN2", target_bir_lowering=False, debug=False,
                   num_devices=N_CORES)
    # x panels per phi-column c: [bt, p, kk*128 + b] with contraction row
    # kappa = k*128 + p inside panel kk//4 (0:XRe, 1:XIm, 2:XRe+XIm)
    xt = [nc.dram_tensor(f"xt{c}", [BT, 128, XKT * 128], bf,
                         kind="ExternalInput") for c in range(2)]
    # weight planes [R, I]: rows kappa=(i,m), cols r-major r*256+o.
    # The Gauss R+I plane is summed on-device (saves startup HBM traffic).
    wt = nc.dram_tensor("wt", [2, KP, 512], bf, kind="ExternalInput")
    out = nc.dram_tensor("out", [BS, OUTW], bf, kind="ExternalOutput")

    with tile.TileContext(nc) as tc:
        with tc.tile_pool(name="wpool", bufs=1) as wpool, \
             tc.tile_pool(name="xpool", bufs=3) as xpool, \
             tc.tile_pool(name="epool", bufs=2) as epool, \
             tc.tile_pool(name="pspool", bufs=1, space="PSUM") as pspool:
            # PE warmup on zeros: ramps the PE p-state during the initial DMA
            # wait so real matmuls start at full clock.
            warm_in = wpool.tile([128, 384], bf, tag="warm_in")
            nc.vector.memset(warm_in[:].bitcast(mybir.dt.uint32), 0)
            # warm_ps shares the g00 ring (retired after warmup, so the
            # spare PSUM banks go to double-buffering g00/g10 instead)
            warm_ps = pspool.tile([128, 512], f32, tag="g00", bufs=PS_DB0)
            for _ in range(N_WARM):
                nc.tensor.matmul(warm_ps[:, 0:256], warm_in[:, :128],
                                 warm_in[:, 128:384], start=True, stop=True)

            # Startup: interleave weight k-tiles with bt0's x panels in PE
            # consumption order (bt0 runs G-major: G1 both c, then G2, G3).
            # The R+I plane never moves over HBM -- DVE sums it from the R
            # and I tiles while the PE chews on G1/G2.
            w_t = [[None] * PKT for _ in range(3)]
            x0_p = [[None] * 3 for _ in range(2)]   # [c][panel] tiles for bt0

            def _w_kgrp(p, k0, nk):
                # one DMA covering k-tiles [k0, k0+nk) of plane p into
                # separate 512-col views of one tile: src rows kappa =
                # k*128 + p_low -> dest [p_low, k, col]
                t = wpool.tile([128, nk * 512], bf, tag=f"w{p}g{k0}",
                               name=f"w{p}g{k0}")
                for k in range(nk):
                    w_t[p][k0 + k] = t[:, k * 512:(k + 1) * 512]
                src = wt[p].copy()
                spart = tuple(src.ap.to_list()[0])
                v = src.ap
                v.clear()
                v.extend([(spart[0], 128), (spart[0] * 128, nk), (1, 512)])
                src.offset = src.offset + k0 * 128 * spart[0]
                nc.sync.dma_start(_rw_ap(t[:], 0, [(512, nk), (1, 512)]), src)

            def wrhs(p, k):
                return w_t[p][k]

            def _x0_dma(c, p):
                # bt0's x panels ride the gpsimd SWDGE queue: Pool generates
                # descriptors in parallel with HWDGE, halving the startup
                # per-DMA overhead serialization.
                x0_p[c][p] = xpool.tile([128, 512], bf, tag=f"x0{c}{p}",
                                        bufs=1, name=f"x0{c}{p}")
                nc.gpsimd.dma_start(x0_p[c][p][:],
                                    xt[c][0][:, p * 512:(p + 1) * 512])

            _x0_dma(0, 0)
            _w_kgrp(0, 0, 1)
            _x0_dma(1, 0)
            _w_kgrp(0, 1, 3)
            _x0_dma(0, 1)
            for k in range(PKT):
                _w_kgrp(1, k, 1)
            _x0_dma(1, 1)

            def _xsum(c, src_re, src_im):
                # on-device XSum panel for the G3 product
                xs = xpool.tile([128, 512], bf, tag=f"xs{c}", bufs=2,
                                name=f"xs{c}")
                nc.vector.tensor_add(xs[:], src_re, src_im)
                return xs

            xs0_c0 = _xsum(0, x0_p[0][0][:], x0_p[0][1][:])
            for k in range(PKT):
                w_t[2][k] = wpool.tile([128, 512], bf, tag=f"w2{k}",
                                       name=f"w2{k}")
                nc.vector.tensor_add(w_t[2][k][:], w_t[0][k], w_t[1][k])
            xs0_c1 = _xsum(1, x0_p[1][0][:], x0_p[1][1][:])

            def _mk_xlhs(panels, xs):
                # panels[c][p] for p in {0,1} are 512-col APs; xs[c] the sum
                def xlhs(c, p, k):
                    if p == 2:
                        return xs[c][:, k * 128:(k + 1) * 128]
                    return panels[c][p][:, k * 128:(k + 1) * 128]
                return xlhs

            cur_xlhs = _mk_xlhs(x0_p, [xs0_c0, xs0_c1])

            for bt in range(BT):
                # Prefetch bt+1's x one full window ahead: DMA + the
                # on-device XSum add, so neither is ever on the PE's path.
                if bt + 1 < BT:
                    nxt = []
                    for c in range(2):
                        t = xpool.tile([128, XKT * 128], bf, tag=f"x{c}",
                                       name=f"x{c}")
                        nc.sync.dma_start(t[:], xt[c][bt + 1])
                        nxt.append(t)
                    panels = [[t[:, 0:512], t[:, 512:1024]] for t in nxt]
                    xs = [_xsum(c, panels[c][0], panels[c][1])
                          for c in range(2)]
                    next_xlhs = _mk_xlhs(panels, xs)

                xlhs = cur_xlhs
                if bt + 1 < BT:
                    cur_xlhs = next_xlhs

                last = bt == BT - 1
                Gs = [[None] * 3, [None] * 3]

                def emit_mm(c, p, half=None, Gs=Gs, xlhs=xlhs):
                    bufs = (PS_DB0 if c == 0 else PS_DB1) if p == 0 else 1
                    g = pspool.tile([128, 512], f32, tag=f"g{c}{p}",
                                    name=f"g{c}{p}", bufs=bufs)
                    for k in range(PKT):
                        rhs = wrhs(p, k) if half is None else \
                            wrhs(p, k)[:, 256 * half:256 * (half + 1)]
                        o = g[:] if half is None else g[:, 0:256]
                        nc.tensor.matmul(o, xlhs(c, p, k), rhs,
                                         start=(k == 0), stop=(k == PKT - 1))
                    Gs[c][p] = g

                g3b = None
                if bt == 0 or last:
                    # G-major: bt0 matches the startup DMA arrival order; the
                    # last bt wants G1/G2 stopped early so the Re-blade
                    # eviction and stores run under the G3 matmuls.
                    for p in range(2):
                        for c in range(2):
                            emit_mm(c, p)
                    emit_mm(0, 2)
                    if last:
                        # G3c1 split: a 256-col PSUM group, then two
                        # SEQUENTIAL 128-col groups sharing the g3b bank
                        # (sequential, not interleaved, so one bank is
                        # legal).  Each stage's Im eviction pipelines under
                        # the next stage's matmuls.
                        emit_mm(1, 2, half=0)
                        g3b = pspool.tile([128, 512], f32, tag="g3b")
                        for h2 in range(2):
                            for k in range(PKT):
                                nc.tensor.matmul(
                                    g3b[:, h2 * 128:(h2 + 1) * 128],
                                    xlhs(1, 2, k),
                                    wrhs(2, k)[:, 256 + h2 * 128:
                                               256 + (h2 + 1) * 128],
                                    start=(k == 0), stop=(k == PKT - 1))
                    else:
                        emit_mm(1, 2)
                else:
                    for c in range(2):
                        for p in range(3):
                            emit_mm(c, p)

                # Eviction.  ScalarE copies PSUM -> bf16 SBUF (DVE reads at
                # most one PSUM operand, and all-bf16 doubles the DVE rate);
                # DVE does the Gauss recombine into t = [Re 512 | Im 512]
                # (r-major halves: A/C = r0/r1 of c0, B/D = r0/r1 of c1) and
                # the inverse-Pauli butterfly into the blade-major stage:
                #   x0 = ReA+ReD  x4 = ReA-ReD  x7 = ImA+ImD  x3 = ImA-ImD
                #   x1 = ReC+ReB  x5 = ReC-ReB  x6 = ImC+ImB  x2 = ImC-ImB
                add, sub = nc.vector.tensor_add, nc.vector.tensor_sub
                inner = (1, 256)
                stage = epool.tile([128, OUTW], bf, tag="stage")
                orows = out[bt * 128:(bt + 1) * 128, 0:OUTW]
                # ACT copies in matmul-stop order so no copy head-of-line
                # blocks an already-stopped G behind it on the in-order ACT
                # engine (stops are G-major on bt0/last, c-major otherwise).
                t_c, u_c = [], []
                gs_c = [[None] * 3, [None] * 3]
                np_copy = 2 if last else 3
                order = [(c, p) for p in range(np_copy) for c in range(2)] \
                    if (bt == 0 or last) else \
                    [(c, p) for c in range(2) for p in range(np_copy)]
                for c, p in order:
                    s = epool.tile([128, 512], bf, tag=f"gs{c}{p}",
                                   name=f"gs{c}{p}")
                    nc.scalar.copy(s[:], Gs[c][p][:])
                    gs_c[c][p] = s
                for c in range(2):
                    gs = gs_c[c]
                    t = epool.tile([128, 1024], bf, tag=f"t{c}", name=f"t{c}")
                    u = epool.tile([128, 512], bf, tag=f"u{c}", name=f"u{c}")
                    nc.vector.tensor_sub(t[:, 0:512], gs[0][:], gs[1][:])
                    if not (last and c == 1):
                        # u_c1 of the last bt is deferred until after the
                        # first Re dual so the dual never queues behind it
                        nc.vector.tensor_add(u[:], gs[0][:], gs[1][:])
                    if not last:
                        nc.vector.tensor_sub(t[:, 512:1024], gs[2][:], u[:])
                    t_c.append(t)
                    u_c.append(u)

                if not last:
                    # Dual-blade butterfly ops; j picks the Re/Im halves.
                    add(_rw_ap(stage[:], 0 * 256, [(1792, 2), inner]),
                        _rw_ap(t_c[0][:], 0, [(512, 2), inner]),
                        _rw_ap(t_c[1][:], 256, [(512, 2), inner]))
                    sub(_rw_ap(stage[:], 4 * 256, [(-256, 2), inner]),
                        _rw_ap(t_c[0][:], 0, [(512, 2), inner]),
                        _rw_ap(t_c[1][:], 256, [(512, 2), inner]))
                    add(_rw_ap(stage[:], 1 * 256, [(1280, 2), inner]),
                        _rw_ap(t_c[0][:], 256, [(512, 2), inner]),
                        _rw_ap(t_c[1][:], 0, [(512, 2), inner]))
                    sub(_rw_ap(stage[:], 5 * 256, [(-768, 2), inner]),
                        _rw_ap(t_c[0][:], 256, [(512, 2), inner]),
                        _rw_ap(t_c[1][:], 0, [(512, 2), inner]))
                    # Steady stores ride gpsimd's SWDGE queue: the sem wait
                    # parks on the otherwise-idle Pool SEQ, so the SP load
                    # queue never stalls behind a store.
                    nc.gpsimd.dma_start(orows, stage[:])
                else:
                    # Re/Im-phased tail: Re blades (j duals (x0,x1), (x4,x5))
                    # need only G1/G2 -- they evict and store while the G3
                    # matmuls still run.  Only the Im blades wait on G3.
                    # The (x4,x5) dual runs on the idle Pool engine: slower
                    # there, but it removes a slot from the serial DVE chain
                    # that gates the final Im store.
                    add(_rw_ap(stage[:], 0, [(256, 2), inner]),
                        _rw_ap(t_c[0][:], 0, [(256, 2), inner]),
                        _rw_ap(t_c[1][:], 256, [(-256, 2), inner]))
                    nc.gpsimd.tensor_sub(
                        _rw_ap(stage[:], 1024, [(256, 2), inner]),
                        _rw_ap(t_c[0][:], 0, [(256, 2), inner]),
                        _rw_ap(t_c[1][:], 256, [(-256, 2), inner]))
                    nc.vector.tensor_add(u_c[1][:], gs_c[1][0][:],
                                         gs_c[1][1][:])
                    nc.sync.dma_start(
                        _rw_ap(orows, 0, [(1024, 2), (1, 512)]),
                        _rw_ap(stage[:], 0, [(1024, 2), (1, 512)]))
                    # Im phase.  c0 full-width (runs under G3c1's matmuls);
                    # c1 in pipelined halves a (cols 0:256 = ImB) and b
                    # (256:512 = ImD, the g3b bank).  Blade singles:
                    #   x2 = ImC-ImB   x6 = ImC+ImB   (a half)
                    #   x3 = ImA-ImD   x7 = ImA+ImD   (b half)
                    s0 = epool.tile([128, 512], bf, tag="gs02", name="gs02")
                    nc.scalar.copy(s0[:], Gs[0][2][:])
                    sa = epool.tile([128, 512], bf, tag="gs12", name="gs12")
                    nc.scalar.copy(sa[:, 0:256], Gs[1][2][:, 0:256])
                    nc.scalar.copy(sa[:, 256:384], g3b[:, 0:128])
                    nc.scalar.copy(sa[:, 384:512], g3b[:, 128:256])
                    nc.vector.tensor_sub(t_c[0][:, 512:1024], s0[:], u_c[0][:])
                    nc.vector.tensor_sub(t_c[1][:, 512:768], sa[:, 0:256],
                                         u_c[1][:, 0:256])
                    sub(stage[:, 512:768], t_c[0][:, 768:1024],
                        t_c[1][:, 512:768])
                    add(stage[:, 1536:1792], t_c[0][:, 768:1024],
                        t_c[1][:, 512:768])
                    nc.scalar.dma_start(
                        _rw_ap(orows, 512, [(1024, 2), inner]),
                        _rw_ap(stage[:], 512, [(1024, 2), inner]))
                    # b halves pipeline: tIm/x3/x7 at 128-col granularity
                    for h2 in range(2):
                        lo, hi = 256 + h2 * 128, 384 + h2 * 128
                        td = slice(768 + h2 * 128, 896 + h2 * 128)
                        ta = slice(512 + h2 * 128, 640 + h2 * 128)
                        nc.vector.tensor_sub(t_c[1][:, td], sa[:, lo:hi],
                                             u_c[1][:, lo:hi])
                        sub(stage[:, 768 + h2 * 128:896 + h2 * 128],
                            t_c[0][:, ta], t_c[1][:, td])
                        add(stage[:, 1792 + h2 * 128:1920 + h2 * 128],
                            t_c[0][:, ta], t_c[1][:, td])
                    nc.sync.dma_start(
                        _rw_ap(orows, 768, [(1024, 2), inner]),
                        _rw_ap(stage[:], 768, [(1024, 2), inner]))
    nc.finalize()
    return nc


def _pauli_parts(v):
    """v[..., 8] -> c0, c1 of shape [..., 2(m/r), 2(reim)]: the c-th column
    (Re, Im) of phi(v).  phi entries: A=P00=(v0+v4)+i(v3+v7),
    B=P01=(v1-v5)+i(v6-v2), C=P10=(v1+v5)+i(v6+v2), D=P11=(v0-v4)+i(v7-v3)."""
    c0 = np.empty(v.shape[:-1] + (2, 2), dtype=v.dtype)
    c1 = np.empty_like(c0)
    v0, v1, v2, v3, v4, v5, v6, v7 = (v[..., a] for a in range(8))
    c0[..., 0, 0] = v0 + v4   # Re A
    c0[..., 0, 1] = v3 + v7   # Im A
    c0[..., 1, 0] = v1 + v5   # Re C
    c0[..., 1, 1] = v6 + v2   # Im C
    c1[..., 0, 0] = v1 - v5   # Re B
    c1[..., 0, 1] = v6 - v2   # Im B
    c1[..., 1, 0] = v0 - v4   # Re D
    c1[..., 1, 1] = v7 - v3   # Im D
    return c0, c1


def _np_bf16():
    return mybir.dt.np(mybir.dt.bfloat16)


def _prep_w(weight):
    """weight [COUT, CIN, 8] -> [2, 512, 512] planes [R, I] of phi(W)[r,m],
    rows (i,m), cols r-major (col = r*256 + o), 0.5 folded.  The Gauss R+I
    plane is summed on-device."""
    w = weight.astype(np.float32)
    cw0, cw1 = _pauli_parts(w)    # cw_m[o, i, r, (re,im)] = phi(W[o,i])[r,m]
    R = np.empty((CIN, 2, 2, COUT), np.float32)   # [(i,m),(r,o)]
    I = np.empty_like(R)
    for m, cm in ((0, cw0), (1, cw1)):
        for r in range(2):
            R[:, m, r, :] = 0.5 * cm[:, :, r, 0].T
            I[:, m, r, :] = 0.5 * cm[:, :, r, 1].T
    Rm = R.reshape(KP, 512)
    Im_ = I.reshape(KP, 512)
    return np.ascontiguousarray(
        np.stack([Rm, Im_], axis=0)).astype(_np_bf16())


def _prep_x(x):
    """x [B, CIN, 8] -> per-core arrays [N_CORES][BT, 128, XKT*128] bf16 for
    c in {0,1}: panels [XRe | XIm | XRe+XIm], device layout [bt, p, kk, b]
    with kappa = k*128 + p, col = kk*128 + b."""
    xf = x.astype(np.float32)
    c0, c1 = _pauli_parts(xf)          # [B, CIN, m, reim]
    outs = []
    for arr in (c0, c1):
        re = arr[..., 0].reshape(B, KP)          # kappa = i*2+m
        im = arr[..., 1].reshape(B, KP)
        panels = np.concatenate([re, im], axis=1)            # col = kk*128+p
        a = panels.reshape(N_CORES, BT, 128, XKT, 128)  # [core, bt, b, kk, p]
        a = a.transpose(0, 1, 4, 3, 2)                  # [core, bt, p, kk, b]
        outs.append(np.ascontiguousarray(
            a.reshape(N_CORES, BT, 128, XKT * 128)).astype(_np_bf16()))
    return outs


def kernel(x, weight, bias, cayley):
    assert x.shape == (B, CIN, NB) and weight.shape == (COUT, CIN, NB)
    if "nc" not in _cached:
        _cached["nc"] = _build_nc()
    nc = _cached["nc"]

    xp = _prep_x(np.asarray(x))
    wp = _prep_w(np.asarray(weight))
    in_maps = [{"xt0": xp[0][c], "xt1": xp[1][c], "wt": wp}
               for c in range(N_CORES)]
    res = run_bass_kernel_spmd(nc, in_maps, core_ids=list(range(N_CORES)))
    out = np.concatenate(
        [np.asarray(res.results[c]["out"]).astype(np.float32)
         for c in range(N_CORES)], axis=0)
    # cols are blade-major (blade*256 + o) -> [B, COUT, NB]
    out = out.reshape(B, NB, COUT).transpose(0, 2, 1)
    out = out + np.asarray(bias, np.float32)[None]
    return np.ascontiguousarray(out.astype(np.float32))
